# revision 7
# baseline (speedup 1.0000x reference)
"""ArcFace head on 8 TRN2 NeuronCores (classifier-parallel / Partial-FC).

out = S * clip(normalize(features) @ normalize(weight).T), with the target
column per row replaced by S * cos(acos(clip(c_tgt)) + M).

Sharding: classes (50000) split 6250/core; features replicated. Each core
computes its (4096, 6250) cosine shard; rows are permuted per core so rows
whose label lands in the core's shard come first, letting the margin update
touch only the first few row-tiles. No collectives needed.

v2 layout: fp16 operand upload (halves input HBM traffic; matmul runs at
the same 1 cyc/row as fp32r/bf16), 13 uniform ~480-wide column tiles, and a
software-pipelined schedule: w-tile 0 is prepped first, the feature
prep chunks are interleaved into column-tile 0's matmul stream (hit-row
group processed last so the margin path is off the critical path), and
w-tile n+1 is loaded/prepped from inside tile n's chain loop. PSUM->SBUF
staging copies rotate across Scalar/Vector/Pool.

Self-contained: hardcodes shapes, builds + compiles a Bass/Tile kernel at
call time, runs it via run_bass_kernel_spmd on cores 0-7, reassembles the
full (4096, 50000) output on the host (pure indexing only).
"""

import math
import sys

import numpy as np

for _p in ("/opt/trn_rl_repo",):
    if _p not in sys.path:
        sys.path.insert(0, _p)

S = 30.0
MARGIN = 0.3
EPS = 1e-7
CLIP_HI = float(np.float32(1.0 - EPS))
CLIP_LO = float(np.float32(-1.0 + EPS))
COS_M = float(np.cos(np.float32(MARGIN)))
SIN_M = float(np.sin(np.float32(MARGIN)))

B, D, C = 4096, 512, 50000
NCORES = 8
CS = C // NCORES          # 6250 classes per core
NTF = 512                 # psum free-dim tile (one PSUM bank of fp32)
KT = D // 128             # 4 contraction tiles
MT = B // 128             # 32 row tiles
STAGE_M = 8               # row tiles per staged output DMA
NGRP = MT // STAGE_M      # 4 row groups
FB = 4                    # f row-tiles per prep chunk


def _col_tiles(CS_):
    """13 uniform even column tiles (482 x5 + 480 x8 for CS=6250)."""
    nt = math.ceil(CS_ / NTF)
    w_lo = (CS_ // nt) // 2 * 2
    rem = CS_ - w_lo * nt
    assert rem % 2 == 0
    n_hi = rem // 2
    tiles, start = [], 0
    for i in range(nt):
        w = w_lo + 2 if i < n_hi else w_lo
        tiles.append((start, w))
        start += w
    assert start == CS_, (start, CS_)
    assert all(256 <= w <= NTF for _, w in tiles)
    return tiles


def _build(B_, CS_, LP):
    """Build the per-core Bass graph. Returns compiled nc."""
    import concourse.bass as bass
    import concourse.tile as tile
    from concourse import bacc, mybir
    from concourse.masks import make_identity

    f32 = mybir.dt.float32
    F16 = mybir.dt.float16
    ALU = mybir.AluOpType
    ACTF = mybir.ActivationFunctionType

    tiles_ = _col_tiles(CS_)
    NT = len(tiles_)
    NMT = LP // 128                     # hit row tiles
    assert NMT <= STAGE_M, "hit rows must fit in row group 0"
    NFC = MT // FB                      # 8 f prep chunks

    nc = bacc.Bacc(
        "TRN2",
        target_bir_lowering=False,
        debug=False,
        enable_asserts=False,
        num_devices=NCORES,
    )

    f_in = nc.dram_tensor("features", [B_, D], F16, kind="ExternalInput").ap()
    w_in = nc.dram_tensor("wshard", [CS_, D], F16, kind="ExternalInput").ap()
    wsel_in = nc.dram_tensor("wsel", [LP, D], F16, kind="ExternalInput").ap()
    labadj_in = nc.dram_tensor("labadj", [128, NMT * NT], f32, kind="ExternalInput").ap()
    iota_in = nc.dram_tensor("iotaf", [128, NTF], f32, kind="ExternalInput").ap()
    out_d = nc.dram_tensor("out", [B_, CS_], f32, kind="ExternalOutput").ap()

    with tile.TileContext(nc) as tc:
        with (
            tc.tile_pool(name="const", bufs=1) as constp,
            tc.tile_pool(name="ftp", bufs=1) as ftp,
            tc.tile_pool(name="fstage", bufs=3) as fstage,
            tc.tile_pool(name="wstage", bufs=2) as wstage,
            tc.tile_pool(name="selstage", bufs=2) as selstage,
            tc.tile_pool(name="sqscr", bufs=2) as sqscr,
            tc.tile_pool(name="normed", bufs=3) as normed,
            tc.tile_pool(name="wtp", bufs=3) as wtp,
            tc.tile_pool(name="stagep", bufs=4) as stagep,
            tc.tile_pool(name="updp", bufs=2) as updp,
            tc.tile_pool(name="smalls", bufs=8) as smalls,
            tc.tile_pool(name="psmm", bufs=6, space="PSUM") as psmm,
            tc.tile_pool(name="pstr", bufs=2, space="PSUM") as pstr,
        ):
            ident_f = constp.tile([128, 128], f32, name="ident_f")
            make_identity(nc, ident_f[:])
            ident = constp.tile([128, 128], F16, name="ident")
            nc.vector.tensor_copy(ident[:], ident_f[:])

            iota_sb = constp.tile([128, NTF], f32, name="iota_sb")
            nc.gpsimd.dma_start(out=iota_sb[:], in_=iota_in[:, :])
            labadj_sb = constp.tile([128, NMT * NT], f32, name="labadj_sb")
            nc.gpsimd.dma_start(out=labadj_sb[:], in_=labadj_in[:, :])
            sdelta = constp.tile([128, NMT], f32, name="sdelta")

            # ---- features: normalize rows (x S), transpose to (D, B) ----
            fT = ftp.tile([128, KT * B_], F16, name="fT")
            fT3 = fT.rearrange("p (k b) -> p k b", k=KT)

            def rownorm_f(src, tag):
                """(128, D) f16 src -> f16 tile = src * S / ||src_row||.
                Square-accum on Vector (keeps Scalar free in the fused
                prologue), sqrt on Scalar, recip+scale on Vector/Pool."""
                scr = sqscr.tile([128, D], f32, name="sq_scr", tag="sq_scr")
                ss = smalls.tile([128, 1], f32, name="ss", tag=f"ss_{tag}")
                nc.scalar.activation(scr[:], src, ACTF.Square, accum_out=ss[:])
                nrm = smalls.tile([128, 1], f32, name="nrm", tag=f"nrm_{tag}")
                nc.scalar.sqrt(nrm[:], ss[:])
                inv = smalls.tile([128, 1], f32, name="inv", tag=f"inv_{tag}")
                nc.vector.reciprocal(inv[:], nrm[:])
                dst = normed.tile([128, D], F16, name="normed_t", tag="normed_t")
                nc.vector.tensor_scalar(
                    out=dst[:], in0=src, scalar1=inv[:], scalar2=float(S),
                    op0=ALU.mult, op1=ALU.mult,
                )
                return dst

            def rownorm_w(src, rows, tag):
                """(rows, D) f16 src -> f16 tile = src / ||src_row||."""
                scr = sqscr.tile([128, D], f32, name="sq_scr", tag="sq_scr")
                ss = smalls.tile([128, 1], f32, name="ss", tag=f"ss_{tag}")
                nc.scalar.activation(scr[:rows], src, ACTF.Square, accum_out=ss[:rows])
                nrm = smalls.tile([128, 1], f32, name="nrm", tag=f"nrm_{tag}")
                nc.scalar.sqrt(nrm[:rows], ss[:rows])
                inv = smalls.tile([128, 1], f32, name="inv", tag=f"inv_{tag}")
                nc.vector.reciprocal(inv[:rows], nrm[:rows])
                dst = normed.tile([128, D], F16, name="normed_t", tag="normed_t")
                nc.vector.tensor_scalar(
                    out=dst[:rows], in0=src, scalar1=inv[:rows], scalar2=1.0,
                    op0=ALU.mult, op1=ALU.mult,
                )
                return dst

            def f_prep(fb):
                """Load + normalize + transpose f row-tiles 4*fb..4*fb+3."""
                fstg = fstage.tile([128, FB * D], F16, name="fstg", tag="fstg")
                fstg3 = fstg.rearrange("p (ci c) -> p ci c", ci=FB)
                nc.sync.dma_start(
                    out=fstg3[:, :, :],
                    in_=f_in[fb * FB * 128:(fb + 1) * FB * 128, :].rearrange(
                        "(ci p) c -> p ci c", p=128
                    ),
                )
                for ci in range(FB):
                    fm = fb * FB + ci
                    fh = rownorm_f(fstg3[:, ci, :], "f")
                    ptr = pstr.tile([128, 512], F16, name="ptr", tag="ptr")
                    for k in range(KT):
                        nc.tensor.transpose(
                            ptr[:, k * 128:(k + 1) * 128],
                            fh[:, k * 128:(k + 1) * 128],
                            ident[:],
                        )
                    ptr3 = ptr.rearrange("p (k x) -> p k x", k=KT)
                    if ci % 2 == 0:
                        nc.scalar.copy(fT3[:, :, fm * 128:(fm + 1) * 128], ptr3[:, :, :])
                    else:
                        nc.vector.tensor_copy(fT3[:, :, fm * 128:(fm + 1) * 128], ptr3[:, :, :])

            def w_load(nt):
                cstart, ncols = tiles_[nt]
                wstg = wstage.tile([128, 4 * D], F16, name="wstg", tag="wstg")
                wstg3 = wstg.rearrange("p (ci c) -> p ci c", ci=4)
                full = ncols // 128
                nc.sync.dma_start(
                    out=wstg3[:, :full, :],
                    in_=w_in[cstart: cstart + full * 128, :].rearrange(
                        "(ci p) c -> p ci c", p=128
                    ),
                )
                if full * 128 < ncols:
                    rr = ncols - full * 128
                    nc.sync.dma_start(
                        out=wstg3[:rr, full, :],
                        in_=w_in[cstart + full * 128: cstart + ncols, :],
                    )
                return wstg3

            def w_make(nt, wstg3):
                cstart, ncols = tiles_[nt]
                nchunks = math.ceil(ncols / 128)
                wT = wtp.tile([128, KT * NTF], F16, name="wT", tag="wT")
                wT3 = wT.rearrange("p (k n) -> p k n", k=KT)
                for ci in range(nchunks):
                    rows = min(128, ncols - ci * 128)
                    wh = rownorm_w(wstg3[:rows, ci, :], rows, "w")
                    ptw = pstr.tile([128, 512], F16, name="ptw", tag="ptr")
                    for k in range(KT):
                        nc.tensor.transpose(
                            ptw[:, k * 128: k * 128 + rows],
                            wh[:rows, k * 128:(k + 1) * 128],
                            ident[:rows, :rows],
                        )
                    ptw3 = ptw.rearrange("p (k x) -> p k x", k=KT)
                    if ci % 2 == 0:
                        nc.scalar.copy(wT3[:, :, ci * 128: ci * 128 + rows], ptw3[:, :, :rows])
                    else:
                        nc.vector.tensor_copy(wT3[:, :, ci * 128: ci * 128 + rows], ptw3[:, :, :rows])
                return wT3

            # ---- tiny path: margin delta per hit row ----
            def tiny(st):
                fs = selstage.tile([128, D], F16, name="fs", tag="fs")
                nc.gpsimd.dma_start(out=fs[:], in_=f_in[st * 128:(st + 1) * 128, :])
                ws = selstage.tile([128, D], F16, name="ws", tag="ws")
                nc.gpsimd.dma_start(out=ws[:], in_=wsel_in[st * 128:(st + 1) * 128, :])

                scrf = sqscr.tile([128, D], f32, name="sq_scr", tag="sq_scr")
                ssf = smalls.tile([128, 1], f32, name="ssf", tag="ssf")
                nc.scalar.activation(scrf[:], fs[:], ACTF.Square, accum_out=ssf[:])
                scrw = sqscr.tile([128, D], f32, name="sq_scr", tag="sq_scr")
                ssw = smalls.tile([128, 1], f32, name="ssw", tag="ssw")
                nc.scalar.activation(scrw[:], ws[:], ACTF.Square, accum_out=ssw[:])

                pscr = sqscr.tile([128, D], f32, name="sq_scr", tag="sq_scr")
                sp = smalls.tile([128, 1], f32, name="sp", tag="sp")
                nc.vector.tensor_mul(pscr[:], fs[:], ws[:])
                nc.vector.reduce_sum(sp[:], pscr[:], mybir.AxisListType.X)
                den = smalls.tile([128, 1], f32, name="den", tag="den")
                nc.vector.tensor_mul(den[:], ssf[:], ssw[:])
                sqd = smalls.tile([128, 1], f32, name="sqd", tag="sqd")
                nc.scalar.sqrt(sqd[:], den[:])
                rinv = smalls.tile([128, 1], f32, name="rinv", tag="rinv")
                nc.vector.reciprocal(rinv[:], sqd[:])
                ct = smalls.tile([128, 1], f32, name="ct", tag="ct")
                nc.vector.tensor_mul(ct[:], sp[:], rinv[:])
                ccl = smalls.tile([128, 1], f32, name="ccl", tag="ccl")
                nc.vector.tensor_scalar(
                    out=ccl[:], in0=ct[:], scalar1=CLIP_HI, scalar2=CLIP_LO,
                    op0=ALU.min, op1=ALU.max,
                )
                c2 = smalls.tile([128, 1], f32, name="c2", tag="c2")
                nc.vector.tensor_mul(c2[:], ccl[:], ccl[:])
                om = smalls.tile([128, 1], f32, name="om", tag="om")
                nc.vector.tensor_scalar(
                    out=om[:], in0=c2[:], scalar1=-1.0, scalar2=1.0,
                    op0=ALU.mult, op1=ALU.add,
                )
                rt = smalls.tile([128, 1], f32, name="rt", tag="rt")
                nc.scalar.sqrt(rt[:], om[:])
                # sdelta = S*(cos(acos(c)+M) - c) = S*(cosM-1)*c - S*sinM*sqrt(1-c^2)
                t1 = smalls.tile([128, 1], f32, name="t1", tag="t1")
                nc.vector.tensor_scalar(
                    out=t1[:], in0=ccl[:], scalar1=float(S * (COS_M - 1.0)),
                    scalar2=None, op0=ALU.mult,
                )
                nc.vector.scalar_tensor_tensor(
                    out=sdelta[:, st:st + 1],
                    in0=rt[:],
                    scalar=float(-S * SIN_M),
                    in1=t1[:],
                    op0=ALU.mult,
                    op1=ALU.add,
                )

            # ---- main loop ----
            out_v = out_d.rearrange("(g m p) c -> g p m c", m=STAGE_M, p=128)
            copy_engines = [
                lambda d, s: nc.scalar.copy(d, s),
                lambda d, s: nc.vector.tensor_copy(d, s),
            ]
            cnt = [0]

            def do_group(nt, wT3, g, hooks):
                cstart, ncols = tiles_[nt]
                stg = stagep.tile([128, STAGE_M * NTF], f32, name="stg", tag="stg")
                stg3 = stg.rearrange("p (m n) -> p m n", m=STAGE_M)
                for mi in range(STAGE_M):
                    mt = g * STAGE_M + mi
                    ps = psmm.tile([128, NTF], f32, name="ps", tag="ps")
                    for k in range(KT):
                        nc.tensor.matmul(
                            ps[:, :ncols],
                            lhsT=fT3[:, k, mt * 128:(mt + 1) * 128],
                            rhs=wT3[:, k, :ncols],
                            start=(k == 0),
                            stop=(k == KT - 1),
                        )
                    dstg = stg3[:, mi, :ncols]
                    if mt < NMT:
                        upd = updp.tile([128, NTF], f32, name="upd", tag="upd")
                        nc.vector.tensor_scalar(
                            out=upd[:, :ncols],
                            in0=iota_sb[:, :ncols],
                            scalar1=labadj_sb[:, mt * NT + nt: mt * NT + nt + 1],
                            scalar2=sdelta[:, mt:mt + 1],
                            op0=ALU.is_equal,
                            op1=ALU.mult,
                        )
                        nc.vector.tensor_add(dstg, ps[:, :ncols], upd[:, :ncols])
                    else:
                        copy_engines[cnt[0] % 2](dstg, ps[:, :ncols])
                        cnt[0] += 1
                    for fn in hooks.get(mi, ()):
                        fn()
                nc.sync.dma_start(
                    out=out_v[g][:, :, cstart: cstart + ncols],
                    in_=stg3[:, :, :ncols],
                )

            # prologue: first w tile + the f chunks needed by row group 1
            wstg0 = w_load(0)
            wT_cur = w_make(0, wstg0)
            f_prep(2)
            f_prep(3)

            # col tile 0: groups ordered 1,2,3,0 so the hit group (0) runs
            # last, after the tiny path has produced sdelta; f chunks are
            # interleaved so each group's fT rows are ready just in time.
            wstg_next = [None]

            def mk(fn, *a):
                return lambda: fn(*a)

            tiny_hooks = {st: [mk(tiny, st)] for st in range(NMT)}

            def w_load_into(nt):
                def _f():
                    wstg_next[0] = w_load(nt)
                return _f

            wT_next = [None]

            def w_make_into(nt):
                def _f():
                    wT_next[0] = w_make(nt, wstg_next[0])
                return _f

            g1 = {mi: list(tiny_hooks.get(mi, [])) for mi in range(STAGE_M)}
            g1.setdefault(2, []).append(mk(f_prep, 4))
            g1.setdefault(5, []).append(mk(f_prep, 5))
            g2 = {1: [mk(f_prep, 6)], 5: [mk(f_prep, 7)]}
            g3 = {0: [w_load_into(1)], 1: [mk(f_prep, 0)],
                  4: [w_make_into(1)], 5: [mk(f_prep, 1)]}
            do_group(0, wT_cur, 1, g1)
            do_group(0, wT_cur, 2, g2)
            do_group(0, wT_cur, 3, g3)
            do_group(0, wT_cur, 0, {})

            for nt in range(1, NT):
                wT_cur = wT_next[0]
                hooks0 = {0: [w_load_into(nt + 1)]} if nt + 1 < NT else {}
                hooks2 = {0: [w_make_into(nt + 1)]} if nt + 1 < NT else {}
                do_group(nt, wT_cur, 0, hooks0)
                do_group(nt, wT_cur, 1, {})
                do_group(nt, wT_cur, 2, hooks2)
                do_group(nt, wT_cur, 3, {})

    nc.compile()
    return nc


def _make_in_maps(features, labels, weight, B_, CS_, n_cores):
    tiles_ = _col_tiles(CS_)
    NT = len(tiles_)
    features = np.ascontiguousarray(features, dtype=np.float16)
    weight16 = np.ascontiguousarray(weight, dtype=np.float16)
    labels_i = np.asarray(labels).astype(np.int64).ravel()
    core_of = labels_i // CS_
    hits = [np.where(core_of == i)[0] for i in range(n_cores)]
    cnt_max = max(len(h) for h in hits)
    LP = max(128, ((cnt_max + 127) // 128) * 128)
    NMT = LP // 128

    iota = np.ascontiguousarray(
        np.broadcast_to(np.arange(NTF, dtype=np.float32), (128, NTF))
    )
    in_maps, perms = [], []
    for i in range(n_cores):
        hit = hits[i]
        perm = np.concatenate([hit, np.where(core_of != i)[0]])
        perms.append(perm)
        wsel = np.ones((LP, D), np.float16)
        wsel[: len(hit)] = weight16[labels_i[hit]]
        labadj = np.full((128, NMT * NT), -1.0, np.float32)
        if len(hit):
            lc = (labels_i[hit] - i * CS_).astype(np.float32)
            r = np.arange(len(hit))
            p, mt = r % 128, r // 128
            for nt, (cstart, _w) in enumerate(tiles_):
                labadj[p, mt * NT + nt] = lc - cstart
        in_maps.append(
            dict(
                features=features[perm],
                wshard=weight16[i * CS_:(i + 1) * CS_],
                wsel=wsel,
                labadj=labadj,
                iotaf=iota,
            )
        )
    return in_maps, perms, LP


_NC_CACHE = {}


def _ensure_ntff_hook():
    """The agent image's antenv lacks axon_hooks; synthesize it so
    run_bass_kernel_spmd(trace=True) can NTFF-profile via the axon .so."""
    import types

    if "antenv.axon_hooks" in sys.modules:
        return
    sys.path.insert(0, "/root/.axon_site")
    from trn_agent_boot.trn_boot import _ntff_profile_via_ctypes

    mod = types.ModuleType("antenv.axon_hooks")
    _state = {"h": None}
    mod.set_axon_ntff_profile_hook = lambda h: _state.__setitem__("h", h)
    mod.get_axon_ntff_profile_hook = lambda: _state["h"]
    sys.modules["antenv.axon_hooks"] = mod
    import antenv

    antenv.axon_hooks = mod
    mod.set_axon_ntff_profile_hook(
        _ntff_profile_via_ctypes("/opt/axon/libaxon_pjrt.so")
    )


def run(features, labels, weight, trace=False, matmul_dtype="float16"):
    """Returns (out, BassKernelResults)."""
    import concourse.bass_utils as bass_utils
    from concourse.bass_utils import run_bass_kernel_spmd

    if trace:
        _ensure_ntff_hook()
        # no S3 in this container; keep artifacts local
        bass_utils.upload_artifacts = lambda tmpdir: tmpdir

    in_maps, perms, LP = _make_in_maps(features, labels, weight, B, CS, NCORES)
    key = (LP,)
    if key not in _NC_CACHE:
        _NC_CACHE[key] = _build(B, CS, LP)
    nc = _NC_CACHE[key]
    res = run_bass_kernel_spmd(
        nc, in_maps, core_ids=list(range(NCORES)), trace=trace
    )
    out = np.empty((B, C), np.float32)
    for i in range(NCORES):
        out[perms[i], i * CS:(i + 1) * CS] = res.results[i]["out"]
    return out, res


def kernel(features, labels, weight):
    out, _ = run(features, labels, weight)
    return out


# revision 8
# speedup vs baseline: 1.1118x; 1.1118x over previous
"""ArcFace head on 8 TRN2 NeuronCores (classifier-parallel / Partial-FC).

out = S * clip(normalize(features) @ normalize(weight).T), with the target
column per row replaced by S * cos(acos(clip(c_tgt)) + M).

Sharding: classes (50000) split 6250/core; features replicated. Each core
computes its (4096, 6250) cosine shard; rows are permuted per core so rows
whose label lands in the core's shard come first, letting the margin update
touch only the first few row-tiles. No collectives needed.

I/O layout: operands are uploaded as fp16 (halves input HBM bytes; fp16
matmul runs at the same 1 cyc/row as fp32r) in pre-tiled (128, chunk, 512)
layouts so every input DMA moves 4-9KB contiguous per partition. Column
tiles are processed in groups of 2-3 sharing one staging buffer, so output
DMAs write 4KB contiguous lines. Feature prep is interleaved between the
row-tile halves of the first column group; the hit-row half of group 0 runs
last so the margin path stays off the critical path.

Self-contained: hardcodes shapes, builds + compiles a Bass/Tile kernel at
call time, runs it via run_bass_kernel_spmd on cores 0-7, reassembles the
full (4096, 50000) output on the host (pure indexing only).
"""

import math
import sys

import numpy as np

for _p in ("/opt/trn_rl_repo",):
    if _p not in sys.path:
        sys.path.insert(0, _p)

S = 30.0
MARGIN = 0.3
EPS = 1e-7
CLIP_HI = float(np.float32(1.0 - EPS))
CLIP_LO = float(np.float32(-1.0 + EPS))
COS_M = float(np.cos(np.float32(MARGIN)))
SIN_M = float(np.sin(np.float32(MARGIN)))

B, D, C = 4096, 512, 50000
NCORES = 8
CS = C // NCORES          # 6250 classes per core
NTF = 512                 # psum free-dim tile (one PSUM bank of fp32)
KT = D // 128             # 4 contraction tiles
MT = B // 128             # 32 row tiles
STAGE_M = 8               # row tiles per staged output DMA
FB = 4                    # f row-tiles per prep chunk
NFC = MT // FB            # 8 f prep chunks
WCH = math.ceil(CS / 128)  # 49 weight chunks of 128 rows (last padded)

# column tiles: 12x512 + 106, grouped (0,1)(2,3)...(10,11,12) so each
# group shares one staging buffer and its output DMA has >=2KB lines
COL_TILES = [(i * 512, 512) for i in range(12)] + [(6144, 106)]
GROUPS = [(0, 1), (2, 3), (4, 5), (6, 7), (8, 9), (10, 11, 12)]
GW_MAX = max(sum(COL_TILES[t][1] for t in g) for g in GROUPS)  # 1130


def _build(B_, CS_, LP):
    """Build the per-core Bass graph. Returns compiled nc."""
    import concourse.bass as bass
    import concourse.tile as tile
    from concourse import bacc, mybir
    from concourse.masks import make_identity

    f32 = mybir.dt.float32
    F16 = mybir.dt.float16
    ALU = mybir.AluOpType
    ACTF = mybir.ActivationFunctionType

    NT = len(COL_TILES)
    NMT = LP // 128                     # hit row tiles
    assert NMT <= STAGE_M, "hit rows must fit in row half 0"

    nc = bacc.Bacc(
        "TRN2",
        target_bir_lowering=False,
        debug=False,
        enable_asserts=False,
        num_devices=NCORES,
    )

    f_in = nc.dram_tensor("features", [128, MT, D], F16, kind="ExternalInput").ap()
    w_in = nc.dram_tensor("wshard", [128, WCH, D], F16, kind="ExternalInput").ap()
    wsel_in = nc.dram_tensor("wsel", [128, NMT, D], F16, kind="ExternalInput").ap()
    labadj_in = nc.dram_tensor("labadj", [128, NMT * NT], f32, kind="ExternalInput").ap()
    iota_in = nc.dram_tensor("iotaf", [128, NTF], f32, kind="ExternalInput").ap()
    out_d = nc.dram_tensor("out", [B_, CS_], f32, kind="ExternalOutput").ap()

    with tile.TileContext(nc) as tc:
        with (
            tc.tile_pool(name="const", bufs=1) as constp,
            tc.tile_pool(name="ftp", bufs=1) as ftp,
            tc.tile_pool(name="fstage", bufs=3) as fstage,
            tc.tile_pool(name="wstage", bufs=2) as wstage,
            tc.tile_pool(name="selstage", bufs=2) as selstage,
            tc.tile_pool(name="sqscr", bufs=2) as sqscr,
            tc.tile_pool(name="normed", bufs=3) as normed,
            tc.tile_pool(name="wtp", bufs=2) as wtp,
            tc.tile_pool(name="stagep", bufs=2) as stagep,
            tc.tile_pool(name="updp", bufs=2) as updp,
            tc.tile_pool(name="smalls", bufs=6) as smalls,
            tc.tile_pool(name="psmm", bufs=5, space="PSUM") as psmm,
            tc.tile_pool(name="pstr", bufs=3, space="PSUM") as pstr,
        ):
            ident_f = constp.tile([128, 128], f32, name="ident_f")
            make_identity(nc, ident_f[:])
            ident = constp.tile([128, 128], F16, name="ident")
            nc.vector.tensor_copy(ident[:], ident_f[:])

            iota_sb = constp.tile([128, NTF], f32, name="iota_sb")
            nc.gpsimd.dma_start(out=iota_sb[:], in_=iota_in[:, :])
            labadj_sb = constp.tile([128, NMT * NT], f32, name="labadj_sb")
            nc.gpsimd.dma_start(out=labadj_sb[:], in_=labadj_in[:, :])
            sdelta = constp.tile([128, NMT], f32, name="sdelta")

            # ---- features: normalize rows (x S), transpose to (D, B) ----
            fT = ftp.tile([128, KT * B_], F16, name="fT")
            fT3 = fT.rearrange("p (k b) -> p k b", k=KT)

            def rownorm_scale(src, scale_imm, tag):
                """(128, D) f16 src -> f16 tile = src * scale / ||row||."""
                scr = sqscr.tile([128, D], f32, name="sq_scr", tag="sq_scr")
                ss = smalls.tile([128, 1], f32, name="ss", tag=f"ss_{tag}")
                nc.scalar.activation(scr[:], src, ACTF.Square, accum_out=ss[:])
                nrm = smalls.tile([128, 1], f32, name="nrm", tag=f"nrm_{tag}")
                nc.scalar.sqrt(nrm[:], ss[:])
                inv = smalls.tile([128, 1], f32, name="inv", tag=f"inv_{tag}")
                nc.vector.reciprocal(inv[:], nrm[:])
                dst = normed.tile([128, D], F16, name="normed_t", tag="normed_t")
                nc.vector.tensor_scalar(
                    out=dst[:], in0=src, scalar1=inv[:], scalar2=float(scale_imm),
                    op0=ALU.mult, op1=ALU.mult,
                )
                return dst

            def f_prep(fb):
                """Load + normalize + transpose f row-tiles FB*fb..FB*fb+3."""
                fstg = fstage.tile([128, FB * D], F16, name="fstg", tag="fstg")
                fstg3 = fstg.rearrange("p (ci c) -> p ci c", ci=FB)
                nc.sync.dma_start(
                    out=fstg3[:, :, :], in_=f_in[:, fb * FB:(fb + 1) * FB, :]
                )
                for ci in range(FB):
                    fm = fb * FB + ci
                    fh = rownorm_scale(fstg3[:, ci, :], S, "f")
                    ptr = pstr.tile([128, 512], F16, name="ptr", tag="ptr")
                    for k in range(KT):
                        nc.tensor.transpose(
                            ptr[:, k * 128:(k + 1) * 128],
                            fh[:, k * 128:(k + 1) * 128],
                            ident[:],
                        )
                    ptr3 = ptr.rearrange("p (k x) -> p k x", k=KT)
                    if ci % 2 == 0:
                        nc.scalar.copy(fT3[:, :, fm * 128:(fm + 1) * 128], ptr3[:, :, :])
                    else:
                        nc.vector.tensor_copy(
                            fT3[:, :, fm * 128:(fm + 1) * 128], ptr3[:, :, :]
                        )

            # ---- weight groups: one load + norm + transpose per group ----
            def wg_chunks(g):
                c0 = COL_TILES[GROUPS[g][0]][0] // 128
                c1 = min(WCH, (COL_TILES[GROUPS[g][-1]][0]
                               + COL_TILES[GROUPS[g][-1]][1] + 127) // 128)
                return c0, c1 - c0

            def wg_load(g):
                c0, nch = wg_chunks(g)
                wstg = wstage.tile([128, 9 * D], F16, name="wstg", tag="wstg")
                wstg3 = wstg.rearrange("p (ci c) -> p ci c", ci=9)
                nc.sync.dma_start(
                    out=wstg3[:, :nch, :], in_=w_in[:, c0:c0 + nch, :]
                )
                return wstg3

            def wg_make(g, wstg3):
                c0, nch = wg_chunks(g)
                wT = wtp.tile([128, KT * 9 * 128], F16, name="wT", tag="wT")
                wT3 = wT.rearrange("p (k n) -> p k n", k=KT)
                for ci in range(nch):
                    wh = rownorm_scale(wstg3[:, ci, :], 1.0, "w")
                    ptw = pstr.tile([128, 512], F16, name="ptw", tag="ptr")
                    for k in range(KT):
                        nc.tensor.transpose(
                            ptw[:, k * 128:(k + 1) * 128],
                            wh[:, k * 128:(k + 1) * 128],
                            ident[:],
                        )
                    ptw3 = ptw.rearrange("p (k x) -> p k x", k=KT)
                    if ci % 2 == 0:
                        nc.vector.tensor_copy(
                            wT3[:, :, ci * 128:(ci + 1) * 128], ptw3[:, :, :]
                        )
                    else:
                        nc.scalar.copy(
                            wT3[:, :, ci * 128:(ci + 1) * 128], ptw3[:, :, :]
                        )
                return wT3

            # ---- tiny path: margin delta per hit row ----
            def tiny(st):
                fs = selstage.tile([128, D], F16, name="fs", tag="fs")
                nc.gpsimd.dma_start(out=fs[:], in_=f_in[:, st, :])
                ws = selstage.tile([128, D], F16, name="ws", tag="ws")
                nc.gpsimd.dma_start(out=ws[:], in_=wsel_in[:, st, :])

                scrf = sqscr.tile([128, D], f32, name="sq_scr", tag="sq_scr")
                ssf = smalls.tile([128, 1], f32, name="ssf", tag="ssf")
                nc.scalar.activation(scrf[:], fs[:], ACTF.Square, accum_out=ssf[:])
                scrw = sqscr.tile([128, D], f32, name="sq_scr", tag="sq_scr")
                ssw = smalls.tile([128, 1], f32, name="ssw", tag="ssw")
                nc.scalar.activation(scrw[:], ws[:], ACTF.Square, accum_out=ssw[:])

                pscr = sqscr.tile([128, D], f32, name="sq_scr", tag="sq_scr")
                sp = smalls.tile([128, 1], f32, name="sp", tag="sp")
                nc.vector.tensor_mul(pscr[:], fs[:], ws[:])
                nc.vector.reduce_sum(sp[:], pscr[:], mybir.AxisListType.X)
                den = smalls.tile([128, 1], f32, name="den", tag="den")
                nc.vector.tensor_mul(den[:], ssf[:], ssw[:])
                sqd = smalls.tile([128, 1], f32, name="sqd", tag="sqd")
                nc.scalar.sqrt(sqd[:], den[:])
                rinv = smalls.tile([128, 1], f32, name="rinv", tag="rinv")
                nc.vector.reciprocal(rinv[:], sqd[:])
                ct = smalls.tile([128, 1], f32, name="ct", tag="ct")
                nc.vector.tensor_mul(ct[:], sp[:], rinv[:])
                ccl = smalls.tile([128, 1], f32, name="ccl", tag="ccl")
                nc.vector.tensor_scalar(
                    out=ccl[:], in0=ct[:], scalar1=CLIP_HI, scalar2=CLIP_LO,
                    op0=ALU.min, op1=ALU.max,
                )
                c2 = smalls.tile([128, 1], f32, name="c2", tag="c2")
                nc.vector.tensor_mul(c2[:], ccl[:], ccl[:])
                om = smalls.tile([128, 1], f32, name="om", tag="om")
                nc.vector.tensor_scalar(
                    out=om[:], in0=c2[:], scalar1=-1.0, scalar2=1.0,
                    op0=ALU.mult, op1=ALU.add,
                )
                rt = smalls.tile([128, 1], f32, name="rt", tag="rt")
                nc.scalar.sqrt(rt[:], om[:])
                # sdelta = S*(cos(acos(c)+M) - c) = S*(cosM-1)*c - S*sinM*sqrt(1-c^2)
                t1 = smalls.tile([128, 1], f32, name="t1", tag="t1")
                nc.vector.tensor_scalar(
                    out=t1[:], in0=ccl[:], scalar1=float(S * (COS_M - 1.0)),
                    scalar2=None, op0=ALU.mult,
                )
                nc.vector.scalar_tensor_tensor(
                    out=sdelta[:, st:st + 1],
                    in0=rt[:],
                    scalar=float(-S * SIN_M),
                    in1=t1[:],
                    op0=ALU.mult,
                    op1=ALU.add,
                )

            # ---- main loop over column-tile groups ----
            out_v = out_d.rearrange("(h m p) c -> h p m c", m=STAGE_M, p=128)
            cnt = [0]

            def do_half(g, wT3, half):
                tiles = GROUPS[g]
                gstart = COL_TILES[tiles[0]][0]
                gw = sum(COL_TILES[t][1] for t in tiles)
                stg = stagep.tile([128, STAGE_M * GW_MAX], f32, name="stg", tag="stg")
                stg3 = stg.rearrange("p (m n) -> p m n", m=STAGE_M)
                for mi in range(STAGE_M):
                    mt = half * STAGE_M + mi
                    soff = 0
                    for t in tiles:
                        cstart, ncols = COL_TILES[t]
                        coff = cstart - COL_TILES[tiles[0]][0]
                        ps = psmm.tile([128, NTF], f32, name="ps", tag="ps")
                        for k in range(KT):
                            nc.tensor.matmul(
                                ps[:, :ncols],
                                lhsT=fT3[:, k, mt * 128:(mt + 1) * 128],
                                rhs=wT3[:, k, coff:coff + ncols],
                                start=(k == 0),
                                stop=(k == KT - 1),
                            )
                        dstg = stg3[:, mi, soff:soff + ncols]
                        if mt < NMT:
                            upd = updp.tile([128, NTF], f32, name="upd", tag="upd")
                            nc.vector.tensor_scalar(
                                out=upd[:, :ncols],
                                in0=iota_sb[:, :ncols],
                                scalar1=labadj_sb[:, mt * NT + t: mt * NT + t + 1],
                                scalar2=sdelta[:, mt:mt + 1],
                                op0=ALU.is_equal,
                                op1=ALU.mult,
                            )
                            nc.vector.tensor_add(dstg, ps[:, :ncols], upd[:, :ncols])
                        elif cnt[0] % 2 == 0:
                            nc.scalar.copy(dstg, ps[:, :ncols])
                            cnt[0] += 1
                        else:
                            nc.vector.tensor_copy(dstg, ps[:, :ncols])
                            cnt[0] += 1
                        soff += ncols
                nc.sync.dma_start(
                    out=out_v[half][:, :, gstart: gstart + gw],
                    in_=stg3[:, :, :gw],
                )

            # prologue: first weight group + first f chunks + tiny path
            wstg0 = wg_load(0)
            wT_cur = wg_make(0, wstg0)
            f_prep(0)
            f_prep(1)
            for st in range(NMT):
                tiny(st)
            f_prep(2)
            f_prep(3)

            # group 0: halves ordered 1,2,3,0 (hit tiles last, after sdelta);
            # remaining f chunks interleaved between halves just in time
            do_half(0, wT_cur, 1)
            f_prep(4)
            f_prep(5)
            do_half(0, wT_cur, 2)
            f_prep(6)
            f_prep(7)
            do_half(0, wT_cur, 3)
            wstg_n = wg_load(1)
            do_half(0, wT_cur, 0)
            wT_next = wg_make(1, wstg_n)

            for g in range(1, len(GROUPS)):
                wT_cur = wT_next
                do_half(g, wT_cur, 0)
                do_half(g, wT_cur, 1)
                if g + 1 < len(GROUPS):
                    wstg_n = wg_load(g + 1)
                do_half(g, wT_cur, 2)
                if g + 1 < len(GROUPS):
                    wT_next = wg_make(g + 1, wstg_n)
                do_half(g, wT_cur, 3)

    nc.compile()
    return nc


def _make_in_maps(features, labels, weight, B_, CS_, n_cores):
    NT = len(COL_TILES)
    features = np.ascontiguousarray(features, dtype=np.float16)
    weight16 = np.ascontiguousarray(weight, dtype=np.float16)
    labels_i = np.asarray(labels).astype(np.int64).ravel()
    core_of = labels_i // CS_
    hits = [np.where(core_of == i)[0] for i in range(n_cores)]
    cnt_max = max(len(h) for h in hits)
    LP = max(128, ((cnt_max + 127) // 128) * 128)
    NMT = LP // 128

    iota = np.ascontiguousarray(
        np.broadcast_to(np.arange(NTF, dtype=np.float32), (128, NTF))
    )
    in_maps, perms = [], []
    for i in range(n_cores):
        hit = hits[i]
        perm = np.concatenate([hit, np.where(core_of != i)[0]])
        perms.append(perm)
        # pre-tiled layouts: x_t[p, chunk, :] = x[chunk*128 + p, :]
        f_t = np.ascontiguousarray(
            features[perm].reshape(MT, 128, D).transpose(1, 0, 2)
        )
        wp = np.ones((WCH * 128, D), np.float16)
        wp[:CS_] = weight16[i * CS_:(i + 1) * CS_]
        w_t = np.ascontiguousarray(wp.reshape(WCH, 128, D).transpose(1, 0, 2))
        wsel = np.ones((LP, D), np.float16)
        wsel[: len(hit)] = weight16[labels_i[hit]]
        wsel_t = np.ascontiguousarray(
            wsel.reshape(NMT, 128, D).transpose(1, 0, 2)
        )
        labadj = np.full((128, NMT * NT), -1.0, np.float32)
        if len(hit):
            lc = (labels_i[hit] - i * CS_).astype(np.float32)
            r = np.arange(len(hit))
            p, mt = r % 128, r // 128
            for nt, (cstart, _w) in enumerate(COL_TILES):
                labadj[p, mt * NT + nt] = lc - cstart
        in_maps.append(
            dict(
                features=f_t,
                wshard=w_t,
                wsel=wsel_t,
                labadj=labadj,
                iotaf=iota,
            )
        )
    return in_maps, perms, LP


_NC_CACHE = {}


def _ensure_ntff_hook():
    """The agent image's antenv lacks axon_hooks; synthesize it so
    run_bass_kernel_spmd(trace=True) can NTFF-profile via the axon .so."""
    import types

    if "antenv.axon_hooks" in sys.modules:
        return
    sys.path.insert(0, "/root/.axon_site")
    from trn_agent_boot.trn_boot import _ntff_profile_via_ctypes

    mod = types.ModuleType("antenv.axon_hooks")
    _state = {"h": None}
    mod.set_axon_ntff_profile_hook = lambda h: _state.__setitem__("h", h)
    mod.get_axon_ntff_profile_hook = lambda: _state["h"]
    sys.modules["antenv.axon_hooks"] = mod
    import antenv

    antenv.axon_hooks = mod
    mod.set_axon_ntff_profile_hook(
        _ntff_profile_via_ctypes("/opt/axon/libaxon_pjrt.so")
    )


def run(features, labels, weight, trace=False, matmul_dtype="float16"):
    """Returns (out, BassKernelResults)."""
    import concourse.bass_utils as bass_utils
    from concourse.bass_utils import run_bass_kernel_spmd

    if trace:
        _ensure_ntff_hook()
        # no S3 in this container; keep artifacts local
        bass_utils.upload_artifacts = lambda tmpdir: tmpdir

    in_maps, perms, LP = _make_in_maps(features, labels, weight, B, CS, NCORES)
    key = (LP,)
    if key not in _NC_CACHE:
        _NC_CACHE[key] = _build(B, CS, LP)
    nc = _NC_CACHE[key]
    res = run_bass_kernel_spmd(
        nc, in_maps, core_ids=list(range(NCORES)), trace=trace
    )
    out = np.empty((B, C), np.float32)
    for i in range(NCORES):
        out[perms[i], i * CS:(i + 1) * CS] = res.results[i]["out"]
    return out, res


def kernel(features, labels, weight):
    out, _ = run(features, labels, weight)
    return out


# revision 10
# speedup vs baseline: 1.2093x; 1.0877x over previous
"""ArcFace head on 8 TRN2 NeuronCores (classifier-parallel / Partial-FC).

out = S * clip(normalize(features) @ normalize(weight).T), with the target
column per row replaced by S * cos(acos(clip(c_tgt)) + M).

Sharding: classes (50000) split 6250/core; features replicated. Each core
computes its (4096, 6250) cosine shard; rows are permuted per core so rows
whose label lands in the core's shard come first, letting the margin update
touch only the first few row-tiles. No collectives needed.

I/O layout: operands are uploaded as fp16 (halves input HBM bytes; fp16
matmul runs at the same 1 cyc/row as fp32r) in pre-tiled (128, chunk, 512)
layouts so every input DMA moves 4-9KB contiguous per partition. Column
tiles are processed in groups of 2-3 sharing one staging buffer, so output
DMAs write 4KB contiguous lines. Feature prep is interleaved between the
row-tile halves of the first column group; the hit-row half of group 0 runs
last so the margin path stays off the critical path.

Self-contained: hardcodes shapes, builds + compiles a Bass/Tile kernel at
call time, runs it via run_bass_kernel_spmd on cores 0-7, reassembles the
full (4096, 50000) output on the host (pure indexing only).
"""

import math
import sys

import numpy as np

for _p in ("/opt/trn_rl_repo",):
    if _p not in sys.path:
        sys.path.insert(0, _p)

S = 30.0
MARGIN = 0.3
EPS = 1e-7
CLIP_HI = float(np.float32(1.0 - EPS))
CLIP_LO = float(np.float32(-1.0 + EPS))
COS_M = float(np.cos(np.float32(MARGIN)))
SIN_M = float(np.sin(np.float32(MARGIN)))

B, D, C = 4096, 512, 50000
NCORES = 8
CS = C // NCORES          # 6250 classes per core
NTF = 512                 # psum free-dim tile (one PSUM bank of fp32)
KT = D // 128             # 4 contraction tiles
MT = B // 128             # 32 row tiles
STAGE_M = 8               # row tiles per staged output DMA
FB = 4                    # f row-tiles per prep chunk
NFC = MT // FB            # 8 f prep chunks
WCH = math.ceil(CS / 128)  # 49 weight chunks of 128 rows (last padded)

# column tiles: 12x512 + 106, grouped (0,1)(2,3)...(10,11,12) so each
# group shares one staging buffer and its output DMA has >=2KB lines
COL_TILES = [(i * 512, 512) for i in range(12)] + [(6144, 106)]
GROUPS = [(0, 1), (2, 3), (4, 5), (6, 7), (8, 9), (10, 11, 12)]
GW_MAX = max(sum(COL_TILES[t][1] for t in g) for g in GROUPS)  # 1130


def _build(B_, CS_, LP):
    """Build the per-core Bass graph. Returns compiled nc."""
    import concourse.bass as bass
    import concourse.tile as tile
    from concourse import bacc, mybir
    from concourse.masks import make_identity

    f32 = mybir.dt.float32
    F16 = mybir.dt.float16
    ALU = mybir.AluOpType
    ACTF = mybir.ActivationFunctionType

    NT = len(COL_TILES)
    NMT = LP // 128                     # hit row tiles
    assert NMT <= STAGE_M, "hit rows must fit in row half 0"

    nc = bacc.Bacc(
        "TRN2",
        target_bir_lowering=False,
        debug=False,
        enable_asserts=False,
        num_devices=NCORES,
    )

    f_in = nc.dram_tensor("features", [128, MT, D], F16, kind="ExternalInput").ap()
    w_in = nc.dram_tensor("wshard", [128, WCH, D], F16, kind="ExternalInput").ap()
    wsel_in = nc.dram_tensor("wsel", [128, NMT, D], F16, kind="ExternalInput").ap()
    labadj_in = nc.dram_tensor("labadj", [128, NMT * NT], f32, kind="ExternalInput").ap()
    iota_in = nc.dram_tensor("iotaf", [128, NTF], f32, kind="ExternalInput").ap()
    out_d = nc.dram_tensor("out", [B_, CS_], f32, kind="ExternalOutput").ap()

    with tile.TileContext(nc) as tc:
        with (
            tc.tile_pool(name="const", bufs=1) as constp,
            tc.tile_pool(name="ftp", bufs=1) as ftp,
            tc.tile_pool(name="fstage", bufs=3) as fstage,
            tc.tile_pool(name="wstage", bufs=2) as wstage,
            tc.tile_pool(name="selstage", bufs=2) as selstage,
            tc.tile_pool(name="sqscr", bufs=2) as sqscr,
            tc.tile_pool(name="normed", bufs=3) as normed,
            tc.tile_pool(name="wtp", bufs=2) as wtp,
            tc.tile_pool(name="stagep", bufs=2) as stagep,
            tc.tile_pool(name="updp", bufs=2) as updp,
            tc.tile_pool(name="smalls", bufs=6) as smalls,
            tc.tile_pool(name="psmm", bufs=5, space="PSUM") as psmm,
            tc.tile_pool(name="pstr", bufs=3, space="PSUM") as pstr,
        ):
            ident_f = constp.tile([128, 128], f32, name="ident_f")
            make_identity(nc, ident_f[:])
            ident = constp.tile([128, 128], F16, name="ident")
            nc.vector.tensor_copy(ident[:], ident_f[:])

            iota_sb = constp.tile([128, NTF], f32, name="iota_sb")
            nc.gpsimd.dma_start(out=iota_sb[:], in_=iota_in[:, :])
            labadj_sb = constp.tile([128, NMT * NT], f32, name="labadj_sb")
            nc.gpsimd.dma_start(out=labadj_sb[:], in_=labadj_in[:, :])
            sdelta = constp.tile([128, NMT], f32, name="sdelta")

            # ---- features: normalize rows (x S), transpose to (D, B) ----
            fT = ftp.tile([128, KT * B_], F16, name="fT")
            fT3 = fT.rearrange("p (k b) -> p k b", k=KT)

            def rownorm_scale(src, scale_imm, tag):
                """(128, D) f16 src -> f16 tile = src * scale / ||row||."""
                scr = sqscr.tile([128, D], f32, name="sq_scr", tag="sq_scr")
                ss = smalls.tile([128, 1], f32, name="ss", tag=f"ss_{tag}")
                nc.scalar.activation(scr[:], src, ACTF.Square, accum_out=ss[:])
                nrm = smalls.tile([128, 1], f32, name="nrm", tag=f"nrm_{tag}")
                nc.scalar.sqrt(nrm[:], ss[:])
                inv = smalls.tile([128, 1], f32, name="inv", tag=f"inv_{tag}")
                nc.vector.reciprocal(inv[:], nrm[:])
                dst = normed.tile([128, D], F16, name="normed_t", tag="normed_t")
                nc.vector.tensor_scalar(
                    out=dst[:], in0=src, scalar1=inv[:], scalar2=float(scale_imm),
                    op0=ALU.mult, op1=ALU.mult,
                )
                return dst

            def f_prep(fb):
                """Load + normalize + transpose f row-tiles FB*fb..FB*fb+3."""
                fstg = fstage.tile([128, FB * D], F16, name="fstg", tag="fstg")
                fstg3 = fstg.rearrange("p (ci c) -> p ci c", ci=FB)
                nc.sync.dma_start(
                    out=fstg3[:, :, :], in_=f_in[:, fb * FB:(fb + 1) * FB, :]
                )
                for ci in range(FB):
                    fm = fb * FB + ci
                    fh = rownorm_scale(fstg3[:, ci, :], S, "f")
                    ptr = pstr.tile([128, 512], F16, name="ptr", tag="ptr")
                    for k in range(KT):
                        nc.tensor.transpose(
                            ptr[:, k * 128:(k + 1) * 128],
                            fh[:, k * 128:(k + 1) * 128],
                            ident[:],
                        )
                    ptr3 = ptr.rearrange("p (k x) -> p k x", k=KT)
                    nc.vector.tensor_copy(
                        fT3[:, :, fm * 128:(fm + 1) * 128], ptr3[:, :, :]
                    )

            # ---- weight groups: one load + norm + transpose per group ----
            def wg_chunks(g):
                c0 = COL_TILES[GROUPS[g][0]][0] // 128
                c1 = min(WCH, (COL_TILES[GROUPS[g][-1]][0]
                               + COL_TILES[GROUPS[g][-1]][1] + 127) // 128)
                return c0, c1 - c0

            def wg_load(g):
                c0, nch = wg_chunks(g)
                wstg = wstage.tile([128, 9 * D], F16, name="wstg", tag="wstg")
                wstg3 = wstg.rearrange("p (ci c) -> p ci c", ci=9)
                nc.sync.dma_start(
                    out=wstg3[:, :nch, :], in_=w_in[:, c0:c0 + nch, :]
                )
                return wstg3

            def wg_make(g, wstg3):
                c0, nch = wg_chunks(g)
                wT = wtp.tile([128, KT * 9 * 128], F16, name="wT", tag="wT")
                wT3 = wT.rearrange("p (k n) -> p k n", k=KT)
                for ci in range(nch):
                    wh = rownorm_scale(wstg3[:, ci, :], 1.0, "w")
                    ptw = pstr.tile([128, 512], F16, name="ptw", tag="ptr")
                    for k in range(KT):
                        nc.tensor.transpose(
                            ptw[:, k * 128:(k + 1) * 128],
                            wh[:, k * 128:(k + 1) * 128],
                            ident[:],
                        )
                    ptw3 = ptw.rearrange("p (k x) -> p k x", k=KT)
                    nc.vector.tensor_copy(
                        wT3[:, :, ci * 128:(ci + 1) * 128], ptw3[:, :, :]
                    )
                return wT3

            # ---- tiny path: margin delta per hit row ----
            def tiny(st):
                fs = selstage.tile([128, D], F16, name="fs", tag="fs")
                nc.gpsimd.dma_start(out=fs[:], in_=f_in[:, st, :])
                ws = selstage.tile([128, D], F16, name="ws", tag="ws")
                nc.gpsimd.dma_start(out=ws[:], in_=wsel_in[:, st, :])

                scrf = sqscr.tile([128, D], f32, name="sq_scr", tag="sq_scr")
                ssf = smalls.tile([128, 1], f32, name="ssf", tag="ssf")
                nc.scalar.activation(scrf[:], fs[:], ACTF.Square, accum_out=ssf[:])
                scrw = sqscr.tile([128, D], f32, name="sq_scr", tag="sq_scr")
                ssw = smalls.tile([128, 1], f32, name="ssw", tag="ssw")
                nc.scalar.activation(scrw[:], ws[:], ACTF.Square, accum_out=ssw[:])

                pscr = sqscr.tile([128, D], f32, name="sq_scr", tag="sq_scr")
                sp = smalls.tile([128, 1], f32, name="sp", tag="sp")
                nc.vector.tensor_mul(pscr[:], fs[:], ws[:])
                nc.vector.reduce_sum(sp[:], pscr[:], mybir.AxisListType.X)
                den = smalls.tile([128, 1], f32, name="den", tag="den")
                nc.vector.tensor_mul(den[:], ssf[:], ssw[:])
                sqd = smalls.tile([128, 1], f32, name="sqd", tag="sqd")
                nc.scalar.sqrt(sqd[:], den[:])
                rinv = smalls.tile([128, 1], f32, name="rinv", tag="rinv")
                nc.vector.reciprocal(rinv[:], sqd[:])
                ct = smalls.tile([128, 1], f32, name="ct", tag="ct")
                nc.vector.tensor_mul(ct[:], sp[:], rinv[:])
                ccl = smalls.tile([128, 1], f32, name="ccl", tag="ccl")
                nc.vector.tensor_scalar(
                    out=ccl[:], in0=ct[:], scalar1=CLIP_HI, scalar2=CLIP_LO,
                    op0=ALU.min, op1=ALU.max,
                )
                c2 = smalls.tile([128, 1], f32, name="c2", tag="c2")
                nc.vector.tensor_mul(c2[:], ccl[:], ccl[:])
                om = smalls.tile([128, 1], f32, name="om", tag="om")
                nc.vector.tensor_scalar(
                    out=om[:], in0=c2[:], scalar1=-1.0, scalar2=1.0,
                    op0=ALU.mult, op1=ALU.add,
                )
                rt = smalls.tile([128, 1], f32, name="rt", tag="rt")
                nc.scalar.sqrt(rt[:], om[:])
                # sdelta = S*(cos(acos(c)+M) - c) = S*(cosM-1)*c - S*sinM*sqrt(1-c^2)
                t1 = smalls.tile([128, 1], f32, name="t1", tag="t1")
                nc.vector.tensor_scalar(
                    out=t1[:], in0=ccl[:], scalar1=float(S * (COS_M - 1.0)),
                    scalar2=None, op0=ALU.mult,
                )
                nc.vector.scalar_tensor_tensor(
                    out=sdelta[:, st:st + 1],
                    in0=rt[:],
                    scalar=float(-S * SIN_M),
                    in1=t1[:],
                    op0=ALU.mult,
                    op1=ALU.add,
                )

            # ---- main loop over column-tile groups ----
            out_v = out_d.rearrange("(h m p) c -> h p m c", m=STAGE_M, p=128)
            cnt = [0]

            def do_half(g, wT3, half):
                tiles = GROUPS[g]
                gstart = COL_TILES[tiles[0]][0]
                gw = sum(COL_TILES[t][1] for t in tiles)
                stg = stagep.tile([128, STAGE_M * GW_MAX], f32, name="stg", tag="stg")
                stg3 = stg.rearrange("p (m n) -> p m n", m=STAGE_M)
                for mi in range(STAGE_M):
                    mt = half * STAGE_M + mi
                    soff = 0
                    for t in tiles:
                        cstart, ncols = COL_TILES[t]
                        coff = cstart - COL_TILES[tiles[0]][0]
                        ps = psmm.tile([128, NTF], f32, name="ps", tag="ps")
                        for k in range(KT):
                            nc.tensor.matmul(
                                ps[:, :ncols],
                                lhsT=fT3[:, k, mt * 128:(mt + 1) * 128],
                                rhs=wT3[:, k, coff:coff + ncols],
                                start=(k == 0),
                                stop=(k == KT - 1),
                            )
                        dstg = stg3[:, mi, soff:soff + ncols]
                        if mt < NMT:
                            upd = updp.tile([128, NTF], f32, name="upd", tag="upd")
                            nc.vector.tensor_scalar(
                                out=upd[:, :ncols],
                                in0=iota_sb[:, :ncols],
                                scalar1=labadj_sb[:, mt * NT + t: mt * NT + t + 1],
                                scalar2=sdelta[:, mt:mt + 1],
                                op0=ALU.is_equal,
                                op1=ALU.mult,
                            )
                            nc.vector.tensor_add(dstg, ps[:, :ncols], upd[:, :ncols])
                        elif cnt[0] % 2 == 0:
                            nc.scalar.copy(dstg, ps[:, :ncols])
                            cnt[0] += 1
                        else:
                            nc.vector.tensor_copy(dstg, ps[:, :ncols])
                            cnt[0] += 1
                        soff += ncols
                if g == len(GROUPS) - 1 and half == 3:
                    for m0 in range(0, STAGE_M, 2):
                        nc.sync.dma_start(
                            out=out_v[half][:, m0:m0 + 2, gstart: gstart + gw],
                            in_=stg3[:, m0:m0 + 2, :gw],
                        )
                else:
                    nc.sync.dma_start(
                        out=out_v[half][:, :, gstart: gstart + gw],
                        in_=stg3[:, :, :gw],
                    )

            # prologue: first weight group + first f chunks + tiny path
            wstg0 = wg_load(0)
            wT_cur = wg_make(0, wstg0)
            f_prep(0)
            f_prep(1)
            f_prep(2)
            f_prep(3)

            # group 0: halves ordered 1,2,3,0 (hit tiles last, after sdelta);
            # remaining f chunks + tiny stages interleaved between halves
            do_half(0, wT_cur, 1)
            f_prep(4)
            f_prep(5)
            for st in range(0, min(2, NMT)):
                tiny(st)
            do_half(0, wT_cur, 2)
            f_prep(6)
            f_prep(7)
            for st in range(2, min(4, NMT)):
                tiny(st)
            do_half(0, wT_cur, 3)
            wstg_n = wg_load(1)
            for st in range(4, NMT):
                tiny(st)
            do_half(0, wT_cur, 0)
            wT_next = wg_make(1, wstg_n)

            for g in range(1, len(GROUPS)):
                wT_cur = wT_next
                do_half(g, wT_cur, 0)
                do_half(g, wT_cur, 1)
                if g + 1 < len(GROUPS):
                    wstg_n = wg_load(g + 1)
                do_half(g, wT_cur, 2)
                if g + 1 < len(GROUPS):
                    wT_next = wg_make(g + 1, wstg_n)
                do_half(g, wT_cur, 3)

    nc.compile()
    return nc


def _make_in_maps(features, labels, weight, B_, CS_, n_cores):
    NT = len(COL_TILES)
    features = np.ascontiguousarray(features, dtype=np.float16)
    weight16 = np.ascontiguousarray(weight, dtype=np.float16)
    labels_i = np.asarray(labels).astype(np.int64).ravel()
    core_of = labels_i // CS_
    hits = [np.where(core_of == i)[0] for i in range(n_cores)]
    cnt_max = max(len(h) for h in hits)
    LP = max(128, ((cnt_max + 127) // 128) * 128)
    NMT = LP // 128

    iota = np.ascontiguousarray(
        np.broadcast_to(np.arange(NTF, dtype=np.float32), (128, NTF))
    )
    in_maps, perms = [], []
    for i in range(n_cores):
        hit = hits[i]
        perm = np.concatenate([hit, np.where(core_of != i)[0]])
        perms.append(perm)
        # pre-tiled layouts: x_t[p, chunk, :] = x[chunk*128 + p, :]
        f_t = np.ascontiguousarray(
            features[perm].reshape(MT, 128, D).transpose(1, 0, 2)
        )
        wp = np.ones((WCH * 128, D), np.float16)
        wp[:CS_] = weight16[i * CS_:(i + 1) * CS_]
        w_t = np.ascontiguousarray(wp.reshape(WCH, 128, D).transpose(1, 0, 2))
        wsel = np.ones((LP, D), np.float16)
        wsel[: len(hit)] = weight16[labels_i[hit]]
        wsel_t = np.ascontiguousarray(
            wsel.reshape(NMT, 128, D).transpose(1, 0, 2)
        )
        labadj = np.full((128, NMT * NT), -1.0, np.float32)
        if len(hit):
            lc = (labels_i[hit] - i * CS_).astype(np.float32)
            r = np.arange(len(hit))
            p, mt = r % 128, r // 128
            for nt, (cstart, _w) in enumerate(COL_TILES):
                labadj[p, mt * NT + nt] = lc - cstart
        in_maps.append(
            dict(
                features=f_t,
                wshard=w_t,
                wsel=wsel_t,
                labadj=labadj,
                iotaf=iota,
            )
        )
    return in_maps, perms, LP


_NC_CACHE = {}


def _ensure_ntff_hook():
    """The agent image's antenv lacks axon_hooks; synthesize it so
    run_bass_kernel_spmd(trace=True) can NTFF-profile via the axon .so."""
    import types

    if "antenv.axon_hooks" in sys.modules:
        return
    sys.path.insert(0, "/root/.axon_site")
    from trn_agent_boot.trn_boot import _ntff_profile_via_ctypes

    mod = types.ModuleType("antenv.axon_hooks")
    _state = {"h": None}
    mod.set_axon_ntff_profile_hook = lambda h: _state.__setitem__("h", h)
    mod.get_axon_ntff_profile_hook = lambda: _state["h"]
    sys.modules["antenv.axon_hooks"] = mod
    import antenv

    antenv.axon_hooks = mod
    mod.set_axon_ntff_profile_hook(
        _ntff_profile_via_ctypes("/opt/axon/libaxon_pjrt.so")
    )


def run(features, labels, weight, trace=False, matmul_dtype="float16"):
    """Returns (out, BassKernelResults)."""
    import concourse.bass_utils as bass_utils
    from concourse.bass_utils import run_bass_kernel_spmd

    if trace:
        _ensure_ntff_hook()
        # no S3 in this container; keep artifacts local
        bass_utils.upload_artifacts = lambda tmpdir: tmpdir

    in_maps, perms, LP = _make_in_maps(features, labels, weight, B, CS, NCORES)
    key = (LP,)
    if key not in _NC_CACHE:
        _NC_CACHE[key] = _build(B, CS, LP)
    nc = _NC_CACHE[key]
    res = run_bass_kernel_spmd(
        nc, in_maps, core_ids=list(range(NCORES)), trace=trace
    )
    out = np.empty((B, C), np.float32)
    for i in range(NCORES):
        out[perms[i], i * CS:(i + 1) * CS] = res.results[i]["out"]
    return out, res


def kernel(features, labels, weight):
    out, _ = run(features, labels, weight)
    return out
